# revision 5
# baseline (speedup 1.0000x reference)
"""Trainium2 Bass kernel for nn_Block_27384711479862 (ConvNeXt-ish metaformer block).

Per-core computation (data parallel over batch B=8 -> 8 cores):
  x: [C=384, N=2304]  (N = 48*48 spatial)
  attention branch (bn1 + qkv + softmax + proj, folded):
      q = qwT.T @ x + qb ; k = kwT.T @ x + kb            (f32r matmuls)
      vT[n, c] = x.T @ vwT                               (no bias; folded via softmax-sum=1)
      s[n, m] = q[:,n].k[:,m];  a = exp(s/sqrt(C)) (no max-sub; logits bounded)
      aT[m, n] = a[n, m] / l[n]   (folded into PE transpose via diag(1/l))
      attn[c, n] = sum_m vT[m, c] aT[m, n]
      x1 = alpha1*x + beta1 + pwT.T @ attn               (beta1 via K=1 ones-matmul)
  mlp branch (bn2 + fc1 + dwconv3x3 + gelu + fc2, folded):
      h = fc1wT.T @ x1 + fc1b  -> stored zero-padded [128, 50, 50] per chunk
      dw = sum_{9 taps} diag(w_tap) @ shifted(h)         (PSUM accumulate)
      g = gelu(dw + dwb)
      out = x1 + fc2wT.T @ g + fc2b                      (fc2b via ones-matmul)
"""
import numpy as np
import ml_dtypes

C = 384
HID = 1536
H = W = 48
N = H * W              # 2304
NC_ = 3                # C chunks of 128
NH = 12                # HID chunks of 128
NRB = 18               # row blocks of 128 queries
EPS = 1e-5
BF16 = ml_dtypes.bfloat16

# n-tiles for the attention side (PSUM bank = 512 f32)
NT5 = [(i * 512, min(512, N - i * 512)) for i in range((N + 511) // 512)]
# n-tiles for the MLP side: 6 tiles of 8 spatial rows (384 cols)
NT6 = [(i * 384, 384) for i in range(6)]
ROWS_PER_TILE = 8
PAD = 50               # padded spatial row stride

_PROG = None           # cached compiled program


def _build_program():
    import concourse.bacc as bacc
    import concourse.bass as bass
    import concourse.mybir as mybir
    import concourse.tile as tile
    from contextlib import ExitStack

    dt = mybir.dt
    AF = mybir.ActivationFunctionType
    ALU = mybir.AluOpType
    f32, f32r, bf16 = dt.float32, dt.float32r, dt.bfloat16

    nc = bacc.Bacc("TRN2", target_bir_lowering=False, debug=False,
                   enable_asserts=False)

    def din(name, shape, d=f32):
        return nc.dram_tensor(name, list(shape), d, kind="ExternalInput").ap()

    x_d = din("x", (C, N))
    qwT_d = din("qwT", (C, C), bf16)
    kwT_d = din("kwT", (C, C), bf16)
    vwT_d = din("vwT", (C, C), bf16)
    pwT_d = din("pwT", (C, C), bf16)
    f1wT_d = din("fc1wT", (C, HID), bf16)
    f2wT_d = din("fc2wT", (HID, C), bf16)
    dww_d = din("dww", (128, NH * 9))
    qb_d = din("qb", (128, NC_))
    kb_d = din("kb", (128, NC_))
    f1b_d = din("fc1b", (128, NH))
    dwb_d = din("dwb", (128, NH))
    al1_d = din("alpha1", (128, NC_))
    b1r_d = din("beta1r", (1, C), bf16)
    f2br_d = din("fc2br", (1, C), bf16)
    iden_d = din("iden", (128, 128), bf16)
    ones_d = din("ones", (1, 512), bf16)
    out_d = nc.dram_tensor("out", [C, N], f32, kind="ExternalOutput").ap()

    inv_sqrt_c = float(1.0 / np.sqrt(np.float32(C)))

    with tile.TileContext(nc) as tc, ExitStack() as top:
        # ---- persistent pools -------------------------------------------
        consts = top.enter_context(tc.tile_pool(name="consts", bufs=1))
        pmm = top.enter_context(tc.tile_pool(name="pmm", bufs=2, space="PSUM"))
        x1p = top.enter_context(tc.tile_pool(name="x1p", bufs=1))

        def load_const(ap, shape, d=f32, tag=None):
            t = consts.tile(list(shape), d, tag=tag, name=tag)
            nc.sync.dma_start(t[:], ap)
            return t

        qb_s = load_const(qb_d, (128, NC_), tag="qb")
        kb_s = load_const(kb_d, (128, NC_), tag="kb")
        f1b_s = load_const(f1b_d, (128, NH), tag="f1b")
        dwb_s = load_const(dwb_d, (128, NH), tag="dwb")
        al1_s = load_const(al1_d, (128, NC_), tag="al1")
        dww_s = load_const(dww_d, (128, NH * 9), tag="dww")
        iden_s = load_const(iden_d, (128, 128), bf16, tag="iden")
        b1r_s = load_const(b1r_d, (1, C), bf16, tag="b1r")
        f2br_s = load_const(f2br_d, (1, C), bf16, tag="f2br")
        ones_s = load_const(ones_d, (1, 512), bf16, tag="ones")

        x1_t = [x1p.tile([128, N], f32, tag=f"x1_{c}", name=f"x1_{c}") for c in range(NC_)]
        x1b_t = [x1p.tile([128, N], bf16, tag=f"x1b_{c}", name=f"x1b_{c}")
                 for c in range(NC_)]

        with ExitStack() as attn_scope:
            wq = attn_scope.enter_context(tc.tile_pool(name="wq", bufs=1))
            xp = attn_scope.enter_context(tc.tile_pool(name="xp", bufs=1))
            qkp = attn_scope.enter_context(tc.tile_pool(name="qkp", bufs=1))
            vTp = attn_scope.enter_context(tc.tile_pool(name="vTp", bufs=1))

            qwT_s = [wq.tile([128, C], bf16, tag=f"qw{k}", name=f"qw{k}") for k in range(NC_)]
            kwT_s = [wq.tile([128, C], bf16, tag=f"kw{k}", name=f"kw{k}") for k in range(NC_)]
            vwT_s = [wq.tile([128, C], bf16, tag=f"vw{k}", name=f"vw{k}") for k in range(NC_)]
            pwT_s = [wq.tile([128, C], bf16, tag=f"pw{k}", name=f"pw{k}") for k in range(NC_)]
            for k in range(NC_):
                sl = slice(k * 128, (k + 1) * 128)
                nc.sync.dma_start(qwT_s[k][:], qwT_d[sl, :])
                nc.sync.dma_start(kwT_s[k][:], kwT_d[sl, :])
                nc.sync.dma_start(vwT_s[k][:], vwT_d[sl, :])
                nc.sync.dma_start(pwT_s[k][:], pwT_d[sl, :])

            x_t = [xp.tile([128, N], f32, tag=f"x_{c}", name=f"x_{c}") for c in range(NC_)]
            for c in range(NC_):
                nc.sync.dma_start(x_t[c][:], x_d[c * 128:(c + 1) * 128, :])

            q_t = [qkp.tile([128, N], bf16, tag=f"q_{c}", name=f"q_{c}") for c in range(NC_)]
            k_t = [qkp.tile([128, N], bf16, tag=f"k_{c}", name=f"k_{c}") for c in range(NC_)]
            xbf_t = [qkp.tile([128, N], bf16, tag=f"xbf_{c}", name=f"xbf_{c}")
                     for c in range(NC_)]
            for c in range(NC_):
                nc.scalar.copy(xbf_t[c][:], x_t[c][:])

            # ---- q, k = w.T @ x + b  (f32r) -----------------------------
            for mc in range(NC_):
                msl = slice(mc * 128, (mc + 1) * 128)
                for (n0, nn) in NT5:
                    for which, wt, bt, dst in (
                        (0, qwT_s, qb_s, q_t), (1, kwT_s, kb_s, k_t)):
                        ps = pmm.tile([128, 512], f32, tag="mm", name="mm")
                        for kc in range(NC_):
                            nc.tensor.matmul(
                                ps[:, :nn],
                                wt[kc][:, msl],
                                xbf_t[kc][:, n0:n0 + nn],
                                start=(kc == 0), stop=(kc == NC_ - 1))
                        nc.scalar.activation(
                            dst[mc][:, n0:n0 + nn], ps[:, :nn],
                            AF.Identity, bias=bt[:, mc:mc + 1])

            # ---- vT[n, c] = x.T @ vwT  (f32r, no bias) ------------------
            vT_t = [vTp.tile([128, C], bf16, tag=f"vT_{b}", name=f"vT_{b}") for b in range(NRB)]
            for nb in range(NRB):
                ps = pmm.tile([128, 512], f32, tag="mm", name="mm")
                for kc in range(NC_):
                    nc.tensor.matmul(
                        ps[:, :C],
                        xbf_t[kc][:, nb * 128:(nb + 1) * 128],
                        vwT_s[kc][:],
                        start=(kc == 0), stop=(kc == NC_ - 1))
                nc.vector.tensor_copy(vT_t[nb][:], ps[:, :C])

            # ---- attention groups (4 row-blocks = 512 queries each) -----
            with ExitStack() as grp_scope:
                ap_ = grp_scope.enter_context(tc.tile_pool(name="ap", bufs=5))
                aTp = grp_scope.enter_context(tc.tile_pool(name="aTp", bufs=1))
                stp = grp_scope.enter_context(tc.tile_pool(name="stp", bufs=6))
                anp = grp_scope.enter_context(tc.tile_pool(name="anp", bufs=4))
                psc = grp_scope.enter_context(
                    tc.tile_pool(name="psc", bufs=2, space="PSUM"))
                ptr = grp_scope.enter_context(
                    tc.tile_pool(name="ptr", bufs=2, space="PSUM"))
                pat = grp_scope.enter_context(
                    tc.tile_pool(name="pat", bufs=2, space="PSUM"))

                groups = [list(range(g, min(g + 4, NRB)))
                          for g in range(0, NRB, 4)]
                for grp in groups:
                    gw = 128 * len(grp)           # group width (queries)
                    g0 = grp[0] * 128
                    aT_t = [aTp.tile([128, gw], bf16, tag=f"aT_{t}", name=f"aT_{t}")
                            for t in range(NRB)]
                    diag_t = []
                    for gi, rb in enumerate(grp):
                        rsl = slice(rb * 128, (rb + 1) * 128)
                        a_t = ap_.tile([128, N], bf16, tag="a", name="a")
                        lsum = stp.tile([128, len(NT5)], f32, tag="ls", name="ls")
                        for ti, (n0, nn) in enumerate(NT5):
                            ps = psc.tile([128, 512], f32, tag="sc", name="sc")
                            for kc in range(NC_):
                                nc.tensor.matmul(
                                    ps[:, :nn], q_t[kc][:, rsl],
                                    k_t[kc][:, n0:n0 + nn],
                                    start=(kc == 0), stop=(kc == NC_ - 1))
                            nc.scalar.activation(
                                a_t[:, n0:n0 + nn], ps[:, :nn], AF.Exp,
                                scale=inv_sqrt_c,
                                accum_out=lsum[:, ti:ti + 1])
                        lt = stp.tile([128, 1], f32, tag="l", name="l")
                        rt = stp.tile([128, 1], f32, tag="r", name="r")
                        dg = stp.tile([128, 128], bf16, tag="dg", name="dg")
                        nc.vector.reduce_sum(lt[:], lsum[:], axis=mybir.AxisListType.X)
                        nc.vector.reciprocal(rt[:], lt[:])
                        nc.vector.tensor_scalar_mul(dg[:], iden_s[:], rt[:])
                        diag_t.append((a_t, dg))

                    # transposes: pairs of row blocks -> one PSUM tile
                    for p0 in range(0, len(grp), 2):
                        pw = 128 * min(2, len(grp) - p0)
                        for t in range(NRB):
                            tp = ptr.tile([128, 256], f32, tag="tr", name="tr")
                            for gi in range(p0, min(p0 + 2, len(grp))):
                                a_t, dg = diag_t[gi]
                                nc.tensor.matmul(
                                    tp[:, (gi - p0) * 128:(gi - p0 + 1) * 128],
                                    a_t[:, t * 128:(t + 1) * 128], dg[:],
                                    start=True, stop=True)
                            nc.vector.tensor_copy(
                                aT_t[t][:, p0 * 128:p0 * 128 + pw], tp[:, :pw])

                    # attn[c, n] = sum_m vT[m, c-chunk] @ aT[m, n]
                    attn_t = []
                    for mc in range(NC_):
                        pa = pat.tile([128, 512], f32, tag="at", name="at")
                        for t in range(NRB):
                            nc.tensor.matmul(
                                pa[:, :gw],
                                vT_t[t][:, mc * 128:(mc + 1) * 128],
                                aT_t[t][:], start=(t == 0), stop=(t == NRB - 1))
                        ab = anp.tile([128, 512], bf16, tag="an", name="an")
                        nc.vector.tensor_copy(ab[:, :gw], pa[:, :gw])
                        attn_t.append(ab)

                    # proj + residual: x1 = alpha1*x + beta1 + pwT.T@attn
                    for mc in range(NC_):
                        msl = slice(mc * 128, (mc + 1) * 128)
                        ps = pmm.tile([128, 512], f32, tag="mm", name="mm")
                        for kc in range(NC_):
                            nc.tensor.matmul(
                                ps[:, :gw], pwT_s[kc][:, msl],
                                attn_t[kc][:, :gw], start=(kc == 0), stop=False)
                        nc.tensor.matmul(
                            ps[:, :gw], b1r_s[:, msl],
                            ones_s[:, :gw],
                            start=False, stop=True)
                        nc.vector.scalar_tensor_tensor(
                            x1_t[mc][:, g0:g0 + gw], x_t[mc][:, g0:g0 + gw],
                            al1_s[:, mc:mc + 1], ps[:, :gw],
                            op0=ALU.mult, op1=ALU.add)
                        nc.scalar.copy(x1b_t[mc][:, g0:g0 + gw],
                                       x1_t[mc][:, g0:g0 + gw])

        # ---- MLP ---------------------------------------------------------
        with ExitStack() as mlp_scope:
            wm = mlp_scope.enter_context(tc.tile_pool(name="wm", bufs=1))
            hp = mlp_scope.enter_context(tc.tile_pool(name="hp", bufs=1))
            gp = mlp_scope.enter_context(tc.tile_pool(name="gp", bufs=2))
            dgp = mlp_scope.enter_context(tc.tile_pool(name="dgp", bufs=1))
            outp = mlp_scope.enter_context(tc.tile_pool(name="outp", bufs=4))
            pdw = mlp_scope.enter_context(
                tc.tile_pool(name="pdw", bufs=2, space="PSUM"))

            f1wT_s = [wm.tile([128, HID], bf16, tag=f"f1w{k}", name=f"f1w{k}") for k in range(NC_)]
            for k in range(NC_):
                nc.sync.dma_start(f1wT_s[k][:], f1wT_d[k * 128:(k + 1) * 128, :])
            f2wT_s = [wm.tile([128, C], bf16, tag=f"f2w{k}", name=f"f2w{k}") for k in range(NH)]
            for k in range(NH):
                nc.sync.dma_start(f2wT_s[k][:], f2wT_d[k * 128:(k + 1) * 128, :])

            # padded h: [128, 50, 50] per HID chunk, borders zeroed
            h_t = [hp.tile([128, PAD * PAD], bf16, tag=f"h_{c}", name=f"h_{c}") for c in range(NH)]
            for c in range(NH):
                hv = h_t[c][:].rearrange("p (y x) -> p y x", y=PAD)
                nc.gpsimd.memset(hv[:, 0, :], 0.0)
                nc.gpsimd.memset(hv[:, PAD - 1, :], 0.0)
                nc.gpsimd.memset(hv[:, :, 0], 0.0)
                nc.gpsimd.memset(hv[:, :, PAD - 1], 0.0)

            # fc1 -> h (padded, bf16, bias via ACT)
            for ti, (n0, nn) in enumerate(NT6):
                y0 = ti * ROWS_PER_TILE
                for hc in range(NH):
                    ps = pmm.tile([128, 512], f32, tag="mm", name="mm")
                    for kc in range(NC_):
                        nc.tensor.matmul(
                            ps[:, :nn],
                            f1wT_s[kc][:, hc * 128:(hc + 1) * 128],
                            x1b_t[kc][:, n0:n0 + nn],
                            start=(kc == 0), stop=(kc == NC_ - 1))
                    dst = h_t[hc][:].rearrange(
                        "p (y x) -> p y x", y=PAD)[
                        :, y0 + 1:y0 + 1 + ROWS_PER_TILE, 1:1 + W]
                    nc.scalar.activation(
                        dst, ps[:, :nn].rearrange("p (y x) -> p y x", y=ROWS_PER_TILE),
                        AF.Identity, bias=f1b_s[:, hc:hc + 1])

            # dwconv diag weights: dg[c][tap] = iden * w  (bf16)
            dwdiag = [[None] * 9 for _ in range(NH)]
            for hc in range(NH):
                for tap in range(9):
                    d = dgp.tile([128, 128], bf16, tag=f"dwd_{hc}_{tap}", name=f"dwd_{hc}_{tap}")
                    nc.vector.tensor_scalar_mul(
                        d[:], iden_s[:], dww_s[:, hc * 9 + tap:hc * 9 + tap + 1])
                    dwdiag[hc][tap] = d

            # dwconv (9 shifted diag matmuls) + gelu -> g ; fc2 + residual
            for ti, (n0, nn) in enumerate(NT6):
                y0 = ti * ROWS_PER_TILE
                g_t = []
                for hc in range(NH):
                    ps = pdw.tile([128, 512], f32, tag="dw", name="dw")
                    hv = h_t[hc][:].rearrange("p (y x) -> p y x", y=PAD)
                    for tap in range(9):
                        dy, dx = divmod(tap, 3)
                        rhs = hv[:, y0 + dy:y0 + dy + ROWS_PER_TILE, dx:dx + W]
                        nc.tensor.matmul(
                            ps[:, :nn].rearrange("p (y x) -> p y x", y=ROWS_PER_TILE),
                            dwdiag[hc][tap][:], rhs,
                            start=(tap == 0), stop=(tap == 8))
                    g = gp.tile([128, 384], bf16, tag=f"g_{hc}", name=f"g_{hc}")
                    nc.scalar.activation(g[:], ps[:, :nn], AF.Gelu,
                                         bias=dwb_s[:, hc:hc + 1])
                    g_t.append(g)
                for mc in range(NC_):
                    msl = slice(mc * 128, (mc + 1) * 128)
                    ps = pmm.tile([128, 512], f32, tag="mm", name="mm")
                    for hc in range(NH):
                        nc.tensor.matmul(ps[:, :nn], f2wT_s[hc][:, msl],
                                         g_t[hc][:], start=(hc == 0), stop=False)
                    nc.tensor.matmul(
                        ps[:, :nn], f2br_s[:, msl],
                        ones_s[:, :nn], start=False, stop=True)
                    ot = outp.tile([128, 384], f32, tag="ot", name="ot")
                    nc.vector.tensor_add(ot[:, :nn], ps[:, :nn],
                                         x1_t[mc][:, n0:n0 + nn])
                    nc.sync.dma_start(out_d[msl, n0:n0 + nn], ot[:, :nn])

    nc.compile()
    return nc


def _fold_inputs(inputs):
    """Host-side weight folding. Returns (shared weight map, per-core x list)."""
    f = np.float32
    g = {k: np.asarray(v, f) for k, v in inputs.items()}
    s1 = g['bn1_g'] / np.sqrt(g['bn1_v'] + EPS)
    t1 = g['bn1_b'] - g['bn1_m'] * s1
    qw = g['q_w'] * s1[None, :]; qb = g['q_w'] @ t1 + g['q_b']
    kw = g['k_w'] * s1[None, :]; kb = g['k_w'] @ t1 + g['k_b']
    vw = g['v_w'] * s1[None, :]
    vb_eff = g['v_w'] @ t1 + g['v_b']
    ls1, ls2 = g['ls1'], g['ls2']
    pw = ls1[:, None] * g['po_w']
    alpha1 = 1.0 + ls1 * s1
    beta1 = ls1 * (g['po_b'] + t1) + pw @ vb_eff
    s2 = g['bn2_g'] / np.sqrt(g['bn2_v'] + EPS)
    t2 = g['bn2_b'] - g['bn2_m'] * s2
    f1w = g['fc1_w'] * s2[None, :]
    f1b = g['fc1_w'] @ t2 + g['fc1_b']
    f2w = ls2[:, None] * g['fc2_w']
    f2b = ls2 * g['fc2_b']
    dww = g['dw_w'].reshape(HID, 9)

    w = {
        'qwT': np.ascontiguousarray(qw.T).astype(BF16),
        'kwT': np.ascontiguousarray(kw.T).astype(BF16),
        'vwT': np.ascontiguousarray(vw.T).astype(BF16),
        'pwT': np.ascontiguousarray(pw.T).astype(BF16),
        'fc1wT': np.ascontiguousarray(f1w.T).astype(BF16),
        'fc2wT': np.ascontiguousarray(f2w.T).astype(BF16),
        'dww': np.ascontiguousarray(dww.reshape(NH, 128, 9).transpose(1, 0, 2)
                                    .reshape(128, NH * 9)),
        'qb': np.ascontiguousarray(qb.reshape(NC_, 128).T),
        'kb': np.ascontiguousarray(kb.reshape(NC_, 128).T),
        'fc1b': np.ascontiguousarray(f1b.reshape(NH, 128).T),
        'dwb': np.ascontiguousarray(g['dw_b'].reshape(NH, 128).T),
        'alpha1': np.ascontiguousarray(alpha1.reshape(NC_, 128).T),
        'beta1r': np.ascontiguousarray(beta1.reshape(1, C)).astype(BF16),
        'fc2br': np.ascontiguousarray(f2b.reshape(1, C)).astype(BF16),
        'iden': np.eye(128, dtype=BF16),
        'ones': np.ones((1, 512), BF16),
    }
    xs = [np.ascontiguousarray(g['x'][b].reshape(C, N)) for b in range(g['x'].shape[0])]
    return w, xs


def get_program():
    global _PROG
    if _PROG is None:
        _PROG = _build_program()
    return _PROG


def kernel(**inputs):
    from concourse.bass_utils import run_bass_kernel_spmd
    nc = get_program()
    w, xs = _fold_inputs(inputs)
    B = len(xs)
    in_maps = [{**w, 'x': xs[b]} for b in range(B)]
    res = run_bass_kernel_spmd(nc, in_maps, list(range(B)))
    out = np.stack([res.results[b]['out'].reshape(C, H, W) for b in range(B)])
    return out.astype(inputs['x'].dtype if hasattr(inputs['x'], 'dtype') else np.float32)
